# revision 1
# baseline (speedup 1.0000x reference)
"""Windowed local self-attention (CrossAttention module with the context-
overwrite bug faithfully reproduced) on 8 Trainium2 NeuronCores.

Full-input contract: kernel(**inputs) takes the unsharded tensors and
returns the full (4, 4096, 1024) output. Internally the 64 independent
windows of 256 tokens are data-parallel sharded 8-per-core; the four
projection weights are broadcast to every core. No collectives needed.

Per-core pipeline (window = 256 tokens, H=16 heads, DH=64):
  X  --PE transpose-->  XT [d, i]
  qT = Wq.T @ X.T   (lhsT=Wq tiles,  rhs=XT)          [o, i]
  kT = Wk.T @ X.T                                      [o, i]
  v  = X @ Wv       (lhsT=XT tiles,  rhs=Wv)           [j, o]
  per head h:
    simT = kT_h.T-free @ qT_h   -> [j, i] in PSUM     (j on partitions)
    es   = exp(0.125 * simT)    (ACT, PSUM->SBUF)
    S    = ones[j,64].T @ es    -> [64, i] broadcast row-sums (PE)
    rS   = 1/S                  (DVE reciprocal)
    o2u  = v_h.T-free @ es      -> [d, i] in PSUM      (AV matmul)
    o2T  = o2u * rS             (DVE, writes stacked [o, i] SBUF)
  Y = o2T.T @ Wo       (lhsT=o2T tiles, rhs=Wo; zero bias added host-side)
All matmul operands are bitcast to float32r: full fp32 bits, 1 cycle/row
on the PE at moving free-dim >= 256 (vs 4 cycles/row for plain float32).
"""

import numpy as np

import concourse.bass as bass
import concourse.mybir as mybir
import concourse.tile as tile
from concourse import bacc, bass_utils
from concourse.bass_interp import get_hw_module
from concourse.masks import make_identity

H = 16
DH = 64
WIN = 256
D = 1024
B = 4
N = 4096
N_CORES = 8
N_WIN_TOTAL = B * N // WIN          # 64
N_WIN = N_WIN_TOTAL // N_CORES      # 8 windows per core
TOK = N_WIN * WIN                   # 2048 token rows per core
SCALE = DH ** -0.5

F32 = mybir.dt.float32
F32R = mybir.dt.float32r


def _r(ap):
    return ap.bitcast(F32R)


def _body(tc, xq, wq, wk, wv, wo, out, n_win):
    nc = tc.nc
    from contextlib import ExitStack

    with ExitStack() as ctx:
        singles = ctx.enter_context(tc.tile_pool(name="singles", bufs=1))
        xpool = ctx.enter_context(tc.tile_pool(name="xpool", bufs=2))
        acts = ctx.enter_context(tc.tile_pool(name="acts", bufs=1))
        heads = ctx.enter_context(tc.tile_pool(name="heads", bufs=2))
        ypool = ctx.enter_context(tc.tile_pool(name="ypool", bufs=2))
        psA = ctx.enter_context(tc.tile_pool(name="psA", bufs=2, space="PSUM"))
        psS = ctx.enter_context(tc.tile_pool(name="psS", bufs=2, space="PSUM"))
        psV = ctx.enter_context(tc.tile_pool(name="psV", bufs=2, space="PSUM"))

        # ---- constants / weights (resident all kernel) ----
        ident_f = singles.tile([128, 128], F32)
        make_identity(nc, ident_f[:])
        ident = singles.tile([128, 128], F32R)
        nc.vector.tensor_copy(ident[:], ident_f[:])
        ones_f = singles.tile([128, 64], F32)
        nc.gpsimd.memset(ones_f[:], 1.0)
        ones64 = singles.tile([128, 64], F32R)
        nc.vector.tensor_copy(ones64[:], ones_f[:])

        # first window's X before the big weight DMAs so transposes start early
        x_first = [xpool.tile([128, D], F32R, tag="x", name=f"x0_{i}") for i in range(2)]
        for tt in range(2):
            nc.sync.dma_start(x_first[tt][:], xq[tt * 128:(tt + 1) * 128, :])

        wsb = {}
        for name, w in (("wq", wq), ("wk", wk), ("wv", wv), ("wo", wo)):
            t = singles.tile([128, 8 * D], F32R, tag=name, name=f"sb_{name}")
            for kt in range(8):
                nc.sync.dma_start(
                    t[:, kt * D:(kt + 1) * D], w[kt * 128:(kt + 1) * 128, :]
                )
            wsb[name] = t

        def emit_transposes(w, x_sb, xt):
            for dt_ in range(8):
                for tt in range(2):
                    pt = psA.tile([128, 128], F32R, tag="acc", name=f"pt_{w}_{dt_}_{tt}")
                    nc.tensor.transpose(
                        pt[:], x_sb[tt][:, dt_ * 128:(dt_ + 1) * 128], ident[:]
                    )
                    nc.vector.tensor_copy(
                        xt[:, dt_ * WIN + tt * 128:dt_ * WIN + tt * 128 + 128], pt[:]
                    )

        def emit_y_group(w, o2T, it, ec):
            row0 = w * WIN
            py = psA.tile([128, 512], F32, tag="acc", name=f"py_{w}_{it}_{ec}")
            for kt2 in range(8):
                nc.tensor.matmul(
                    py[:],
                    o2T[:, kt2 * WIN + it * 128:kt2 * WIN + (it + 1) * 128],
                    wsb["wo"][:, kt2 * D + ec * 512:kt2 * D + (ec + 1) * 512],
                    start=(kt2 == 0),
                    stop=(kt2 == 7),
                )
            y_sb = ypool.tile([128, 512], F32, tag="y", name=f"y_{w}_{it}_{ec}")
            nc.vector.tensor_copy(y_sb[:], py[:])
            nc.sync.dma_start(
                out[row0 + it * 128:row0 + (it + 1) * 128, ec * 512:(ec + 1) * 512],
                y_sb[:],
            )

        prev = None  # (o2T of previous window)
        for w in range(n_win):
            row0 = w * WIN
            if w == 0:
                x_sb = x_first
            else:
                x_sb = [xpool.tile([128, D], F32R, tag="x", name=f"x_{w}_{i}") for i in range(2)]
                for tt in range(2):
                    nc.sync.dma_start(
                        x_sb[tt][:], xq[row0 + tt * 128:row0 + (tt + 1) * 128, :]
                    )

            xt = acts.tile([128, 8 * WIN], F32R, tag="xt", name=f"xt_{w}")
            if prev is None:
                emit_transposes(w, x_sb, xt)
            else:
                # interleave: 4 transposes, then one Y group of previous window
                for chunk in range(4):
                    for dt_ in range(2 * chunk, 2 * chunk + 2):
                        for tt in range(2):
                            pt = psA.tile([128, 128], F32R, tag="acc",
                                          name=f"pt_{w}_{dt_}_{tt}")
                            nc.tensor.transpose(
                                pt[:], x_sb[tt][:, dt_ * 128:(dt_ + 1) * 128], ident[:]
                            )
                            nc.vector.tensor_copy(
                                xt[:, dt_ * WIN + tt * 128:dt_ * WIN + tt * 128 + 128],
                                pt[:],
                            )
                    emit_y_group(w - 1, prev, chunk // 2, chunk % 2)

            # ---- qT, kT [128, 2048] ----
            proj = {}
            for pname, wname in (("qT", "wq"), ("kT", "wk")):
                dst = acts.tile([128, 8 * WIN], F32R, tag=pname, name=f"{pname}_{w}")
                wtile = wsb[wname]
                for ot in range(8):
                    pq = psA.tile([128, WIN], F32, tag="acc", name=f"pq_{w}_{pname}_{ot}")
                    for kt in range(8):
                        nc.tensor.matmul(
                            pq[:],
                            wtile[:, kt * D + ot * 128:kt * D + (ot + 1) * 128],
                            xt[:, kt * WIN:(kt + 1) * WIN],
                            start=(kt == 0),
                            stop=(kt == 7),
                        )
                    nc.vector.tensor_copy(dst[:, ot * WIN:(ot + 1) * WIN], pq[:])
                proj[pname] = dst
            qT, kT = proj["qT"], proj["kT"]

            # ---- v natural [128 j, 2048] ----
            v_sb = acts.tile([128, 2 * D], F32R, tag="v", name=f"v_{w}")
            for jt in range(2):
                for oc in range(2):
                    pv = psA.tile([128, 512], F32, tag="acc", name=f"pv_{w}_{jt}_{oc}")
                    for kt in range(8):
                        nc.tensor.matmul(
                            pv[:],
                            xt[:, kt * WIN + jt * 128:kt * WIN + (jt + 1) * 128],
                            wsb["wv"][:, kt * D + oc * 512:kt * D + (oc + 1) * 512],
                            start=(kt == 0),
                            stop=(kt == 7),
                        )
                    nc.vector.tensor_copy(
                        v_sb[:, jt * D + oc * 512:jt * D + (oc + 1) * 512], pv[:]
                    )

            # ---- attention: head pairs, software-pipelined ----
            o2T = acts.tile([128, 8 * WIN], F32R, tag="o2T", name=f"o2T_{w}")

            es_t = [None] * H

            def emit_sim(h):
                prow = (h % 2) * 64
                ocol = (h // 2) * WIN
                qh = qT[prow:prow + 64, ocol:ocol + WIN]
                kh = kT[prow:prow + 64, ocol:ocol + WIN]
                ps_sim = psS.tile([128, 512], F32, tag="sim", name=f"sim_{w}_{h}")
                for jt in range(2):
                    nc.tensor.matmul(
                        ps_sim[:, jt * WIN:(jt + 1) * WIN],
                        kh[:, jt * 128:(jt + 1) * 128],
                        qh,
                        start=True,
                        stop=True,
                    )
                e = heads.tile([128, 512], F32R, tag="es", name=f"es_{w}_{h}")
                nc.scalar.activation(
                    e[:], ps_sim[:], mybir.ActivationFunctionType.Exp, scale=SCALE
                )
                es_t[h] = e

            def emit_pair(p):
                for h in (2 * p, 2 * p + 1):
                    s_ps = psV.tile([64, WIN], F32, tag="s", bufs=2,
                                    name=f"s_{w}_{h}")
                    av_ps = psV.tile([64, WIN], F32, tag="av", bufs=2,
                                     name=f"av_{w}_{h}")
                    for jt in range(2):
                        nc.tensor.matmul(
                            s_ps[:],
                            ones64[:, 0:64],
                            es_t[h][:, jt * WIN:(jt + 1) * WIN],
                            start=(jt == 0),
                            stop=(jt == 1),
                        )
                    for jt in range(2):
                        nc.tensor.matmul(
                            av_ps[:],
                            v_sb[:, jt * D + h * DH:jt * D + (h + 1) * DH],
                            es_t[h][:, jt * WIN:(jt + 1) * WIN],
                            start=(jt == 0),
                            stop=(jt == 1),
                        )
                    s_sb = heads.tile([64, WIN], F32, tag="s_sb",
                                      name=f"ssb_{w}_{h}")
                    nc.vector.tensor_copy(s_sb[:], s_ps[:])
                    rs = heads.tile([64, WIN], F32, tag="rs", name=f"rs_{w}_{h}")
                    nc.vector.reciprocal_approx_fast(rs[:], s_sb[:])
                    r0 = (h % 2) * 64
                    nc.vector.tensor_mul(
                        o2T[r0:r0 + 64, p * WIN:(p + 1) * WIN], av_ps[:], rs[:]
                    )
                    es_t[h] = None

            emit_sim(0)
            emit_sim(1)
            for p in range(1, 8):
                emit_sim(2 * p)
                emit_sim(2 * p + 1)
                emit_pair(p - 1)
            emit_pair(7)

            prev = o2T

        for chunk in range(4):
            emit_y_group(n_win - 1, prev, chunk // 2, chunk % 2)


_CACHE = {}


def _build(n_win=N_WIN):
    key = n_win
    if key in _CACHE:
        return _CACHE[key]
    tok = n_win * WIN
    nc = bacc.Bacc(
        "TRN2", target_bir_lowering=False, debug=False, num_devices=N_CORES
    )
    xq = nc.dram_tensor("xq", [tok, D], F32R, kind="ExternalInput").ap()
    wq = nc.dram_tensor("Wq", [D, D], F32R, kind="ExternalInput").ap()
    wk = nc.dram_tensor("Wk", [D, D], F32R, kind="ExternalInput").ap()
    wv = nc.dram_tensor("Wv", [D, D], F32R, kind="ExternalInput").ap()
    wo = nc.dram_tensor("Wo", [D, D], F32R, kind="ExternalInput").ap()
    out = nc.dram_tensor("out", [tok, D], F32, kind="ExternalOutput").ap()
    with tile.TileContext(nc) as tc:
        _body(tc, xq, wq, wk, wv, wo, out, n_win)
    nc.compile()
    nc.m = get_hw_module(nc.m)
    _CACHE[key] = nc
    return nc


def run(query, Wq, Wk, Wv, Wo, bo, n_win=N_WIN, **spmd_kwargs):
    nc = _build(n_win)
    tok = n_win * WIN
    q2 = np.ascontiguousarray(np.asarray(query, dtype=np.float32).reshape(-1, D))
    weights = {
        "Wq": np.ascontiguousarray(np.asarray(Wq, np.float32)),
        "Wk": np.ascontiguousarray(np.asarray(Wk, np.float32)),
        "Wv": np.ascontiguousarray(np.asarray(Wv, np.float32)),
        "Wo": np.ascontiguousarray(np.asarray(Wo, np.float32)),
    }
    in_maps = []
    for c in range(N_CORES):
        m = {"xq": q2[c * TOK:c * TOK + tok]}
        m.update(weights)
        in_maps.append(m)
    res = bass_utils.run_bass_kernel_spmd(
        nc, in_maps, core_ids=list(range(N_CORES)), **spmd_kwargs
    )
    outs = [res.results[c]["out"] for c in range(N_CORES)]
    return outs, res


def kernel(query, context, Wq, Wk, Wv, Wo, bo):
    outs, _ = run(query, Wq, Wk, Wv, Wo, bo)
    y = np.concatenate(outs, axis=0).reshape(B, N, D)
    bo = np.asarray(bo, np.float32)
    if bo.any():
        y = y + bo  # bias is structurally zero for this problem; host-add keeps exactness
    return y.astype(np.float32)



# revision 8
# speedup vs baseline: 1.3527x; 1.3527x over previous
"""Windowed local self-attention (CrossAttention module with the context-
overwrite bug faithfully reproduced) on 8 Trainium2 NeuronCores.

Full-input contract: kernel(**inputs) takes the unsharded tensors and
returns the full (4, 4096, 1024) output. The 64 independent windows of
256 tokens are data-parallel sharded 8-per-core; the four projection
weights are broadcast to every core. No collectives needed.

v2 design (vs the fp32r baseline):
  * everything bf16 on-chip (PSUM accumulation stays fp32). fp32r and
    bf16 both stream 1 cycle/row on the PE, but bf16 halves the
    LDWEIGHTS time, which was stalling every 256-col matmul (~63ns/mm).
  * X is transposed host-side per window -> the 16 PE transposes + DVE
    drains per window disappear.
  * per-PAIR attention: sim for heads (2p, 2p+1) lands in two PSUM
    tiles [128, 512] (jt-major), one exp covers both heads, the row-sum
    matmul is shared (ones[128,64] @ es: 2 mm of 512 cols per pair).
  * PSUM drains are spread across engines: qT/kT -> GpSimd(Pool),
    v -> Scalar(ACT copy), y/recip/normalize-mul -> DVE.
  * cross-window software pipeline: SAV(6,7) and the four Y groups of
    window w-1 are woven between the projection accumulations of
    window w, so the PE never waits on the exp/recip chain.

Per-core timeline per window (PE cycles @2.4GHz, bf16 1 cyc/col):
  qT 16384 + kT 16384 + v 16384 + y 16384 + att 16*(512+256+512)
  ~= 85.5k cycles ~= 35.6us; 8 windows ~= 285us + overheads.
"""

import numpy as np
import ml_dtypes

import concourse.bass as bass
import concourse.mybir as mybir
import concourse.tile as tile
from concourse import bacc, bass_utils
from concourse.bass_interp import get_hw_module

H = 16
DH = 64
WIN = 256
D = 1024
B = 4
N = 4096
N_CORES = 8
N_WIN_TOTAL = B * N // WIN          # 64
N_WIN = N_WIN_TOTAL // N_CORES      # 8 windows per core
SCALE = DH ** -0.5

F32 = mybir.dt.float32
BF16 = mybir.dt.bfloat16
EXP = mybir.ActivationFunctionType.Exp
COPY = mybir.ActivationFunctionType.Copy


def _body(tc, xt_d, wq, wk, wv, wo, out, n_win):
    nc = tc.nc
    from contextlib import ExitStack

    with ExitStack() as ctx:
        singles = ctx.enter_context(tc.tile_pool(name="singles", bufs=1))
        xpool = ctx.enter_context(tc.tile_pool(name="xpool", bufs=2))
        qkpool = ctx.enter_context(tc.tile_pool(name="qkpool", bufs=2))
        vpool = ctx.enter_context(tc.tile_pool(name="vpool", bufs=2))
        epool = ctx.enter_context(tc.tile_pool(name="epool", bufs=3))
        rspool = ctx.enter_context(tc.tile_pool(name="rspool", bufs=2))
        opool = ctx.enter_context(tc.tile_pool(name="opool", bufs=2))
        ypool = ctx.enter_context(tc.tile_pool(name="ypool", bufs=2))
        psP = ctx.enter_context(tc.tile_pool(name="psP", bufs=2, space="PSUM"))
        psS = ctx.enter_context(tc.tile_pool(name="psS", bufs=3, space="PSUM"))
        psSum = ctx.enter_context(tc.tile_pool(name="psSum", bufs=1, space="PSUM"))
        psV = ctx.enter_context(tc.tile_pool(name="psV", bufs=2, space="PSUM"))

        # ---- constants / weights (resident all kernel) ----
        ones64 = singles.tile([128, 64], BF16)
        nc.gpsimd.memset(ones64[:], 1.0)

        # first window's X before the big weight DMAs so compute starts early
        xt0 = xpool.tile([128, 8 * WIN], BF16, tag="xt", name="xt_0")
        for dt_ in range(8):
            nc.sync.dma_start(
                xt0[:, dt_ * WIN:(dt_ + 1) * WIN],
                xt_d[dt_ * 128:(dt_ + 1) * 128, :],
            )

        wsb = {}
        for name, w in (("wq", wq), ("wk", wk), ("wv", wv), ("wo", wo)):
            t = singles.tile([128, 8 * D], BF16, tag=name, name=f"sb_{name}")
            for kt in range(8):
                nc.sync.dma_start(
                    t[:, kt * D:(kt + 1) * D], w[kt * 128:(kt + 1) * 128, :]
                )
            wsb[name] = t

        # per-window live tiles, filled by the emit helpers below
        cur = {}

        def emit_x_dma(w):
            t = xpool.tile([128, 8 * WIN], BF16, tag="xt", name=f"xt_{w}")
            for dt_ in range(8):
                nc.sync.dma_start(
                    t[:, dt_ * WIN:(dt_ + 1) * WIN],
                    xt_d[w * D + dt_ * 128:w * D + dt_ * 128 + 128, :],
                )
            return t

        def emit_proj_tile(w, a, wname, dstname):
            """qT/kT ot-pair `a`: accumulate [128, 512] then drain.
            GPSIMD cannot read PSUM, so drains go to DVE (qT) / ACT (kT)."""
            xt = cur["xt"]
            pq = psP.tile([128, 512], F32, tag="acc", name=f"p_{dstname}_{w}_{a}")
            for ot2 in range(2):
                ot = 2 * a + ot2
                for kt in range(8):
                    nc.tensor.matmul(
                        pq[:, ot2 * WIN:(ot2 + 1) * WIN],
                        wsb[wname][:, kt * D + ot * 128:kt * D + (ot + 1) * 128],
                        xt[:, kt * WIN:(kt + 1) * WIN],
                        start=(kt == 0),
                        stop=(kt == 7),
                    )
            dst = cur[dstname]
            if dstname == "qT":
                nc.vector.tensor_copy(dst[:, 2 * a * WIN:(2 * a + 2) * WIN], pq[:])
            else:
                nc.scalar.activation(
                    dst[:, 2 * a * WIN:(2 * a + 2) * WIN], pq[:], COPY
                )

        def emit_v_tile(w, jt, oc):
            xt = cur["xt"]
            pv = psP.tile([128, 512], F32, tag="acc", name=f"pv_{w}_{jt}_{oc}")
            for kt in range(8):
                nc.tensor.matmul(
                    pv[:],
                    xt[:, kt * WIN + jt * 128:kt * WIN + (jt + 1) * 128],
                    wsb["wv"][:, kt * D + oc * 512:kt * D + (oc + 1) * 512],
                    start=(kt == 0),
                    stop=(kt == 7),
                )
            v_sb = cur["v"]
            nc.scalar.activation(
                v_sb[:, jt * D + oc * 512:jt * D + (oc + 1) * 512], pv[:], COPY
            )

        def emit_sim(w, p):
            """sim + exp for head pair (2p, 2p+1); returns the es tile.

            HW quirk: a single PSUM tile must only receive matmuls from ONE
            PE quadrant position, so each h2 (tile_position (h2*64, 0)) gets
            its own PSUM tile; the exp then scatters back into the jt-major
            es layout (es[jp, jt*512 + h2*256 + i]) via a strided dst AP."""
            qT, kT = cur["qT"], cur["kT"]
            es = epool.tile([128, 1024], BF16, tag="es", name=f"es_{w}_{p}")
            for h2 in range(2):
                prow = h2 * 64
                T = psS.tile([128, 512], F32, tag="sim", name=f"sim_{w}_{p}_{h2}")
                for jt in range(2):
                    nc.tensor.matmul(
                        T[:, jt * WIN:(jt + 1) * WIN],
                        kT[prow:prow + 64,
                           p * WIN + jt * 128:p * WIN + jt * 128 + 128],
                        qT[prow:prow + 64, p * WIN:(p + 1) * WIN],
                        start=True,
                        stop=True,
                    )
                dst = es[:].rearrange("p (a b) -> p a b", a=2)[
                    :, :, h2 * WIN:(h2 + 1) * WIN]
                src = T[:].rearrange("p (a b) -> p a b", a=2)
                nc.scalar.activation(dst, src, EXP, scale=SCALE)
            return es

        def emit_sav(w, p, es, v_sb, o2T):
            """row-sums, reciprocal, AV, and normalize for pair p."""
            s_ps = psSum.tile([64, 512], F32, tag="s", name=f"s_{w}_{p}")
            for jt in range(2):
                nc.tensor.matmul(
                    s_ps[:],
                    ones64[:, 0:64],
                    es[:, jt * 512:(jt + 1) * 512],
                    start=(jt == 0),
                    stop=(jt == 1),
                )
            rs = rspool.tile([64, 512], F32, tag="rs", name=f"rs_{w}_{p}")
            nc.vector.reciprocal_approx_fast(rs[:], s_ps[:])
            av = psV.tile([64, 512], F32, tag="av", name=f"av_{w}_{p}")
            for h2 in range(2):
                h = 2 * p + h2
                for jt in range(2):
                    nc.tensor.matmul(
                        av[:, h2 * WIN:(h2 + 1) * WIN],
                        v_sb[:, jt * D + h * DH:jt * D + (h + 1) * DH],
                        es[:, jt * 512 + h2 * WIN:jt * 512 + (h2 + 1) * WIN],
                        start=(jt == 0),
                        stop=(jt == 1),
                    )
            for h2 in range(2):
                nc.vector.tensor_mul(
                    o2T[h2 * 64:h2 * 64 + 64, p * WIN:(p + 1) * WIN],
                    av[:, h2 * WIN:(h2 + 1) * WIN],
                    rs[:, h2 * WIN:(h2 + 1) * WIN],
                )

        def emit_y_group(w, o2T_w, g):
            it, ec = g // 2, g % 2
            row0 = w * WIN
            py = psP.tile([128, 512], F32, tag="acc", name=f"py_{w}_{g}")
            for kt2 in range(8):
                nc.tensor.matmul(
                    py[:],
                    o2T_w[:, kt2 * WIN + it * 128:kt2 * WIN + (it + 1) * 128],
                    wsb["wo"][:, kt2 * D + ec * 512:kt2 * D + (ec + 1) * 512],
                    start=(kt2 == 0),
                    stop=(kt2 == 7),
                )
            y_sb = ypool.tile([128, 512], F32, tag="y", name=f"y_{w}_{g}")
            nc.vector.tensor_copy(y_sb[:], py[:])
            nc.sync.dma_start(
                out[row0 + it * 128:row0 + (it + 1) * 128,
                    ec * 512:(ec + 1) * 512],
                y_sb[:],
            )

        prev = None  # (w-1, o2T, es6, es7) of previous window

        for w in range(n_win):
            cur["xt"] = xt0 if w == 0 else cur.pop("xt_next")
            cur["qT"] = qkpool.tile([128, 8 * WIN], BF16, tag="qT", name=f"qT_{w}")
            cur["kT"] = qkpool.tile([128, 8 * WIN], BF16, tag="kT", name=f"kT_{w}")
            cur["v"] = vpool.tile([128, 2 * D], BF16, tag="v", name=f"v_{w}")
            cur["o2T"] = opool.tile([128, 8 * WIN], BF16, tag="o2T", name=f"o2T_{w}")

            # -- qT/kT/v accumulations, with window w-1's attention tail and
            #    output projection woven in between the PSUM-tile groups --
            tail = []
            if prev is not None:
                pw, pv_sb, po2T, pes6, pes7 = prev
                tail = [
                    lambda: emit_sav(pw, 6, pes6, pv_sb, po2T),
                    lambda: emit_sav(pw, 7, pes7, pv_sb, po2T),
                    lambda: emit_y_group(pw, po2T, 0),
                    lambda: emit_y_group(pw, po2T, 1),
                    lambda: emit_y_group(pw, po2T, 2),
                    lambda: emit_y_group(pw, po2T, 3),
                ]

            for a in range(4):
                emit_proj_tile(w, a, "wq", "qT")
                if a < len(tail):
                    tail[a]()
            for a in range(4):
                emit_proj_tile(w, a, "wk", "kT")
                if a + 4 < len(tail):
                    tail[a + 4]()

            if w + 1 < n_win:
                cur["xt_next"] = emit_x_dma(w + 1)

            for jt in range(2):
                for oc in range(2):
                    emit_v_tile(w, jt, oc)

            # -- attention pairs 0..7, SAV lagging one pair; 6 and 7 deferred
            #    into window w+1's projection phase --
            es_prev = emit_sim(w, 0)
            es_list = [es_prev]
            for p in range(1, 8):
                es_list.append(emit_sim(w, p))
                if p >= 2:
                    emit_sav(w, p - 2, es_list[p - 2], cur["v"], cur["o2T"])
            emit_sav(w, 5, es_list[5], cur["v"], cur["o2T"])

            prev = (w, cur["v"], cur["o2T"], es_list[6], es_list[7])

        pw, pv_sb, po2T, pes6, pes7 = prev
        emit_sav(pw, 6, pes6, pv_sb, po2T)
        emit_sav(pw, 7, pes7, pv_sb, po2T)
        for g in range(4):
            emit_y_group(pw, po2T, g)


_CACHE = {}


def _build(n_win=N_WIN):
    key = n_win
    if key in _CACHE:
        return _CACHE[key]
    nc = bacc.Bacc(
        "TRN2", target_bir_lowering=False, debug=False, num_devices=N_CORES
    )
    xt_d = nc.dram_tensor("xt", [n_win * D, WIN], BF16, kind="ExternalInput").ap()
    wq = nc.dram_tensor("Wq", [D, D], BF16, kind="ExternalInput").ap()
    wk = nc.dram_tensor("Wk", [D, D], BF16, kind="ExternalInput").ap()
    wv = nc.dram_tensor("Wv", [D, D], BF16, kind="ExternalInput").ap()
    wo = nc.dram_tensor("Wo", [D, D], BF16, kind="ExternalInput").ap()
    out = nc.dram_tensor("out", [n_win * WIN, D], F32, kind="ExternalOutput").ap()
    with tile.TileContext(nc) as tc:
        _body(tc, xt_d, wq, wk, wv, wo, out, n_win)
    nc.compile()
    nc.m = get_hw_module(nc.m)
    _CACHE[key] = nc
    return nc


def _bf16(a):
    return np.ascontiguousarray(np.asarray(a, np.float32)).astype(ml_dtypes.bfloat16)


def run(query, Wq, Wk, Wv, Wo, bo, n_win=N_WIN, **spmd_kwargs):
    nc = _build(n_win)
    # host-side: window-transpose X so the device never transposes
    q3 = np.asarray(query, np.float32).reshape(-1, WIN, D)      # (64, 256, 1024)
    qt = np.ascontiguousarray(q3.transpose(0, 2, 1)).astype(ml_dtypes.bfloat16)
    weights = {
        "Wq": _bf16(Wq), "Wk": _bf16(Wk), "Wv": _bf16(Wv), "Wo": _bf16(Wo),
    }
    in_maps = []
    for c in range(N_CORES):
        m = {"xt": np.ascontiguousarray(
            qt[c * N_WIN:c * N_WIN + n_win].reshape(n_win * D, WIN))}
        m.update(weights)
        in_maps.append(m)
    res = bass_utils.run_bass_kernel_spmd(
        nc, in_maps, core_ids=list(range(N_CORES)), **spmd_kwargs
    )
    outs = [res.results[c]["out"] for c in range(N_CORES)]
    return outs, res


def kernel(query, context, Wq, Wk, Wv, Wo, bo):
    outs, _ = run(query, Wq, Wk, Wv, Wo, bo)
    y = np.concatenate(outs, axis=0).reshape(B, N, D)
    bo = np.asarray(bo, np.float32)
    if bo.any():
        y = y + bo  # bias is structurally zero for this problem; host-add keeps exactness
    return y.astype(np.float32)


# revision 9
# speedup vs baseline: 1.3785x; 1.0190x over previous
"""v3: window-PAIR batched projections + consolidated DMAs.

Differences vs v2:
  * qT/kT computed for two windows at once: moving operand is
    xt2[:, kt*512 : +512] (both windows' 256 tokens), so every
    projection matmul streams 512 cols and the bf16 LDWEIGHTS
    (~135ns) hides completely under the 213ns stream.
  * weights arrive as 2 consolidated DMAs each (4 kt-blocks per DMA,
    strided dst via rearrange), X as 1 DMA per window pair; the SP
    issue rate (~630ns/DMA) stops dominating the startup.
"""

import numpy as np
import ml_dtypes

import concourse.bass as bass
import concourse.mybir as mybir
import concourse.tile as tile
from concourse import bacc, bass_utils
from concourse.bass_interp import get_hw_module

H = 16
DH = 64
WIN = 256
D = 1024
B = 4
N = 4096
N_CORES = 8
N_WIN_TOTAL = B * N // WIN          # 64
N_WIN = N_WIN_TOTAL // N_CORES      # 8 windows per core
SCALE = DH ** -0.5

F32 = mybir.dt.float32
BF16 = mybir.dt.bfloat16
EXP = mybir.ActivationFunctionType.Exp
COPY = mybir.ActivationFunctionType.Copy


def _body(tc, xt_d, wq, wk, wv, wo, out, n_win):
    nc = tc.nc
    from contextlib import ExitStack

    n_pair = n_win // 2

    with ExitStack() as ctx:
        singles = ctx.enter_context(tc.tile_pool(name="singles", bufs=1))
        xpool = ctx.enter_context(tc.tile_pool(name="xpool", bufs=2))
        qkpool = ctx.enter_context(tc.tile_pool(name="qkpool", bufs=2))
        vpool = ctx.enter_context(tc.tile_pool(name="vpool", bufs=2))
        epool = ctx.enter_context(tc.tile_pool(name="epool", bufs=3))
        rspool = ctx.enter_context(tc.tile_pool(name="rspool", bufs=2))
        opool = ctx.enter_context(tc.tile_pool(name="opool", bufs=2))
        ypool = ctx.enter_context(tc.tile_pool(name="ypool", bufs=2))
        psP = ctx.enter_context(tc.tile_pool(name="psP", bufs=2, space="PSUM"))
        psS = ctx.enter_context(tc.tile_pool(name="psS", bufs=3, space="PSUM"))
        psSum = ctx.enter_context(tc.tile_pool(name="psSum", bufs=1, space="PSUM"))
        psV = ctx.enter_context(tc.tile_pool(name="psV", bufs=2, space="PSUM"))

        ones64 = singles.tile([128, 64], BF16)
        nc.gpsimd.memset(ones64[:], 1.0)

        def emit_x_dma(pr):
            """Two DMAs (one per window): pair pr -> xt2[:, kt*512 + u*256 + i].
            (DMA APs are limited to 3 dims, so the pair can't be one DMA.)"""
            t = xpool.tile([128, 8 * 2 * WIN], BF16, tag="xt", name=f"xt_{pr}")
            for u in range(2):
                w = 2 * pr + u
                src = xt_d[w * D:(w + 1) * D, :].rearrange(
                    "(a p) b -> p a b", p=128)
                dst = t[:].rearrange("p (a c) -> p a c", a=8)[
                    :, :, u * WIN:(u + 1) * WIN]
                nc.sync.dma_start(dst, src)
            return t

        # pair-0 X first, then weights in 2-block chunks, kt-major order
        xt2_first = emit_x_dma(0)
        wsb = {}
        for name, w in (("wq", wq), ("wk", wk), ("wv", wv), ("wo", wo)):
            t = singles.tile([128, 8 * D], BF16, tag=name, name=f"sb_{name}")
            for half in range(2):
                src = w[half * 512:(half + 1) * 512, :].rearrange(
                    "(a p) b -> p a b", p=128)
                dst = t[:, half * 4 * D:(half + 1) * 4 * D].rearrange(
                    "p (a b) -> p a b", a=4)
                nc.sync.dma_start(dst, src)
            wsb[name] = t

        cur = {}

        def emit_qk_tile(pr, ot, wname, dstname):
            """one ot block for BOTH windows: 8 accum matmuls of 512 cols."""
            xt2 = cur["xt2"]
            pq = psP.tile([128, 512], F32, tag="acc", name=f"p_{dstname}_{pr}_{ot}")
            for kt in range(8):
                nc.tensor.matmul(
                    pq[:],
                    wsb[wname][:, kt * D + ot * 128:kt * D + (ot + 1) * 128],
                    xt2[:, kt * 512:(kt + 1) * 512],
                    start=(kt == 0),
                    stop=(kt == 7),
                )
            dst = cur[dstname]
            if dstname == "qT":
                nc.vector.tensor_copy(dst[:, ot * 512:(ot + 1) * 512], pq[:])
            else:
                nc.scalar.activation(dst[:, ot * 512:(ot + 1) * 512], pq[:], COPY)

        def emit_v_tile(pr, u, jt, oc):
            xt2 = cur["xt2"]
            pv = psP.tile([128, 512], F32, tag="acc", name=f"pv_{pr}_{u}_{jt}_{oc}")
            for kt in range(8):
                nc.tensor.matmul(
                    pv[:],
                    xt2[:, kt * 512 + u * WIN + jt * 128:
                        kt * 512 + u * WIN + (jt + 1) * 128],
                    wsb["wv"][:, kt * D + oc * 512:kt * D + (oc + 1) * 512],
                    start=(kt == 0),
                    stop=(kt == 7),
                )
            v_sb = cur["v"][u]
            nc.scalar.activation(
                v_sb[:, jt * D + oc * 512:jt * D + (oc + 1) * 512], pv[:], COPY
            )

        def emit_sim(pr, u, p):
            """sim + exp for pair p of window u; PSUM tile per h2 quadrant."""
            qT, kT = cur["qT"], cur["kT"]
            es = epool.tile([128, 1024], BF16, tag="es", name=f"es_{pr}_{u}_{p}")
            q0 = p * 512 + u * WIN
            for h2 in range(2):
                prow = h2 * 64
                T = psS.tile([128, 512], F32, tag="sim",
                             name=f"sim_{pr}_{u}_{p}_{h2}")
                for jt in range(2):
                    nc.tensor.matmul(
                        T[:, jt * WIN:(jt + 1) * WIN],
                        kT[prow:prow + 64, q0 + jt * 128:q0 + jt * 128 + 128],
                        qT[prow:prow + 64, q0:q0 + WIN],
                        start=True,
                        stop=True,
                    )
                dst = es[:].rearrange("p (a b) -> p a b", a=2)[
                    :, :, h2 * WIN:(h2 + 1) * WIN]
                src = T[:].rearrange("p (a b) -> p a b", a=2)
                nc.scalar.activation(dst, src, EXP, scale=SCALE)
            return es

        def emit_sav(tag, p, es, v_sb, o2T):
            s_ps = psSum.tile([64, 512], F32, tag="s", name=f"s_{tag}_{p}")
            for jt in range(2):
                nc.tensor.matmul(
                    s_ps[:],
                    ones64[:, 0:64],
                    es[:, jt * 512:(jt + 1) * 512],
                    start=(jt == 0),
                    stop=(jt == 1),
                )
            rs = rspool.tile([64, 512], F32, tag="rs", name=f"rs_{tag}_{p}")
            nc.vector.reciprocal_approx_fast(rs[:], s_ps[:])
            av = psV.tile([64, 512], F32, tag="av", name=f"av_{tag}_{p}")
            for h2 in range(2):
                h = 2 * p + h2
                for jt in range(2):
                    nc.tensor.matmul(
                        av[:, h2 * WIN:(h2 + 1) * WIN],
                        v_sb[:, jt * D + h * DH:jt * D + (h + 1) * DH],
                        es[:, jt * 512 + h2 * WIN:jt * 512 + (h2 + 1) * WIN],
                        start=(jt == 0),
                        stop=(jt == 1),
                    )
            for h2 in range(2):
                nc.vector.tensor_mul(
                    o2T[h2 * 64:h2 * 64 + 64, p * WIN:(p + 1) * WIN],
                    av[:, h2 * WIN:(h2 + 1) * WIN],
                    rs[:, h2 * WIN:(h2 + 1) * WIN],
                )

        def emit_y_group(w, o2T_w, g):
            it, ec = g // 2, g % 2
            row0 = w * WIN
            py = psP.tile([128, 512], F32, tag="acc", name=f"py_{w}_{g}")
            for kt2 in range(8):
                nc.tensor.matmul(
                    py[:],
                    o2T_w[:, kt2 * WIN + it * 128:kt2 * WIN + (it + 1) * 128],
                    wsb["wo"][:, kt2 * D + ec * 512:kt2 * D + (ec + 1) * 512],
                    start=(kt2 == 0),
                    stop=(kt2 == 7),
                )
            y_sb = ypool.tile([128, 512], F32, tag="y", name=f"y_{w}_{g}")
            nc.vector.tensor_copy(y_sb[:], py[:])
            nc.sync.dma_start(
                out[row0 + it * 128:row0 + (it + 1) * 128,
                    ec * 512:(ec + 1) * 512],
                y_sb[:],
            )

        def emit_att(pr, u, tail):
            """attention for window u of pair pr; SAV lags 2 pairs; the last
            two SAVs are returned as deferred closures. `tail` items (from
            the previous window) are woven between pair slots."""
            w = 2 * pr + u
            v_sb, o2T = cur["v"][u], cur["o2T"][u]
            es_list = []
            deferred = []
            ti = 0
            for p in range(8):
                es_list.append(emit_sim(pr, u, p))
                if ti < len(tail):
                    tail[ti]()
                    ti += 1
                if p >= 2:
                    emit_sav(f"{pr}_{u}", p - 2, es_list[p - 2], v_sb, o2T)
            while ti < len(tail):
                tail[ti]()
                ti += 1
            emit_sav(f"{pr}_{u}", 5, es_list[5], v_sb, o2T)

            def d1(p=6):
                emit_sav(f"{pr}_{u}", p, es_list[p], v_sb, o2T)

            def d2(p=7):
                emit_sav(f"{pr}_{u}", p, es_list[p], v_sb, o2T)

            yg = [lambda g=g: emit_y_group(w, o2T, g) for g in range(4)]
            return [d1, d2] + yg

        tail = []  # deferred SAV(6,7) + y groups of the previous window
        for pr in range(n_pair):
            cur["xt2"] = xt2_first if pr == 0 else cur.pop("xt2_next")
            cur["qT"] = qkpool.tile([128, 16 * WIN], BF16, tag="qT", name=f"qT_{pr}")
            cur["kT"] = qkpool.tile([128, 16 * WIN], BF16, tag="kT", name=f"kT_{pr}")
            cur["v"] = [vpool.tile([128, 2 * D], BF16, tag=f"v{u}",
                                   name=f"v_{pr}_{u}") for u in range(2)]
            cur["o2T"] = [opool.tile([128, 8 * WIN], BF16, tag=f"o2T{u}",
                                     name=f"o2T_{pr}_{u}") for u in range(2)]

            # P1: projections, weaving in the previous window's tail
            ti = 0
            for ot in range(8):
                emit_qk_tile(pr, ot, "wq", "qT")
                if ti < len(tail):
                    tail[ti]()
                    ti += 1
            for ot in range(8):
                emit_qk_tile(pr, ot, "wk", "kT")
                if ti < len(tail):
                    tail[ti]()
                    ti += 1
            tail = tail[ti:]

            if pr + 1 < n_pair:
                cur["xt2_next"] = emit_x_dma(pr + 1)

            for u in range(2):
                for jt in range(2):
                    for oc in range(2):
                        emit_v_tile(pr, u, jt, oc)

            # attention: window u=0 (weave remaining tail), then u=1 (weave
            # window 0's deferred SAV+y)
            tail0 = emit_att(pr, 0, tail)
            tail = emit_att(pr, 1, tail0)

        for fn in tail:
            fn()


_CACHE = {}


def _build(n_win=N_WIN):
    key = n_win
    if key in _CACHE:
        return _CACHE[key]
    nc = bacc.Bacc(
        "TRN2", target_bir_lowering=False, debug=False, num_devices=N_CORES
    )
    xt_d = nc.dram_tensor("xt", [n_win * D, WIN], BF16, kind="ExternalInput").ap()
    wq = nc.dram_tensor("Wq", [D, D], BF16, kind="ExternalInput").ap()
    wk = nc.dram_tensor("Wk", [D, D], BF16, kind="ExternalInput").ap()
    wv = nc.dram_tensor("Wv", [D, D], BF16, kind="ExternalInput").ap()
    wo = nc.dram_tensor("Wo", [D, D], BF16, kind="ExternalInput").ap()
    out = nc.dram_tensor("out", [n_win * WIN, D], F32, kind="ExternalOutput").ap()
    with tile.TileContext(nc) as tc:
        _body(tc, xt_d, wq, wk, wv, wo, out, n_win)
    nc.compile()
    nc.m = get_hw_module(nc.m)
    _CACHE[key] = nc
    return nc


def _bf16(a):
    return np.ascontiguousarray(np.asarray(a, np.float32)).astype(ml_dtypes.bfloat16)


def run(query, Wq, Wk, Wv, Wo, bo, n_win=N_WIN, **spmd_kwargs):
    nc = _build(n_win)
    q3 = np.asarray(query, np.float32).reshape(-1, WIN, D)      # (64, 256, 1024)
    qt = np.ascontiguousarray(q3.transpose(0, 2, 1)).astype(ml_dtypes.bfloat16)
    weights = {
        "Wq": _bf16(Wq), "Wk": _bf16(Wk), "Wv": _bf16(Wv), "Wo": _bf16(Wo),
    }
    in_maps = []
    for c in range(N_CORES):
        m = {"xt": np.ascontiguousarray(
            qt[c * N_WIN:c * N_WIN + n_win].reshape(n_win * D, WIN))}
        m.update(weights)
        in_maps.append(m)
    res = bass_utils.run_bass_kernel_spmd(
        nc, in_maps, core_ids=list(range(N_CORES)), **spmd_kwargs
    )
    outs = [res.results[c]["out"] for c in range(N_CORES)]
    return outs, res


def kernel(query, context, Wq, Wk, Wv, Wo, bo):
    outs, _ = run(query, Wq, Wk, Wv, Wo, bo)
    y = np.concatenate(outs, axis=0).reshape(B, N, D)
    bo = np.asarray(bo, np.float32)
    if bo.any():
        y = y + bo  # bias is structurally zero for this problem; host-add keeps exactness
    return y.astype(np.float32)
